# revision 7
# baseline (speedup 1.0000x reference)
"""GuidedFilter (2-angle box guided filter) on 8 trn2 NeuronCores.

Math: for each stage s in {0, 1}:
    X <- X + box_s(y - X) / N_s
with box_0 = 17(rows) x 5(cols) ones kernel, box_1 = 5 x 17, zero-padded,
N_s the matching box filter of ones (separable: N_s = v_s(r) * h_s(c)).

Implementation per core (rows sharded, 256 rows/core, halo 10):
  3 independent row-chunks (128/128/60 source rows, stride 108).
  - g0 = rowwise cumsum(y - X)            (stock tensor_tensor_scan, DVE)
  - w0 = 5-tap window sums via shifted diffs of g0 (+ edge scale fixes)
  - C1 psum = V0w^T @ w0                  (TensorE; vertical 17-tap sum,
                                           normalizers folded into weights)
  - g1 = g0 - cumsum(C1)                  (custom DVE op: fused residual+scan)
  - w1 = 17-tap window sums of g1
  - psum += V1w^T @ w1                    (C1 + C2 accumulated in psum)
  - out = X + psum                        (ACT copy psum->sbuf, GPSIMD add)
"""

import sys

if "/opt/trn_rl_repo" not in sys.path:
    sys.path.insert(0, "/opt/trn_rl_repo")

import numpy as np

M_DIM = N = 2048
NCORES = 8
RPC = 256          # rows per core
HALO = 10
SRC_ROWS = RPC + 2 * HALO          # 276
CHUNKS = [(0, 128), (108, 128), (216, 60)]   # (local row start, rows)
OUT_LO = 10
G_PAD = 9
GW = G_PAD + N                     # 2057

_CACHE = {}


def _register_custom_op():
    from concourse.dve_spec import Spec, Src0, Src1, scan, AluOp, lower
    import concourse.dve_ops as dops
    from concourse.dve_uop import DveOpSpec

    name = "SUB_CUMSUM_GF"
    for op in dops.OPS:
        if op.name == name:
            return op
    spec = Spec(
        body=Src0 - scan(AluOp.ADD, Src1),
        reference=lambda in0, in1: in0 - np.cumsum(in1, axis=-1),
    )
    op = dops.DveOp(name, spec, subdim=False, uops_sha={})
    dops.OPS.append(op)
    dops.CUSTOM_DVE_SPECS[name] = spec
    dops._SUB_OPCODE_FOR_NAME[name] = max(dops._SUB_OPCODE_FOR_NAME.values()) + 1
    opc = dops.get_dve_sub_opcode(name)
    for ver in ("v3", "v4"):
        s = DveOpSpec(name=name, opcode=opc, uops=lower(spec, ver=ver), rd1_en=True)
        op.uops_sha[ver] = s.sha(ver)
    return op


def _build_program():
    from concourse import bacc
    import concourse.mybir as mybir
    from concourse.tile import TileContext

    OP = _register_custom_op()
    f32 = mybir.dt.float32
    alu = mybir.AluOpType

    nc = bacc.Bacc("TRN2", target_bir_lowering=False)
    Xc = nc.dram_tensor("Xc", (SRC_ROWS, N), f32, kind="ExternalInput")
    yc = nc.dram_tensor("yc", (SRC_ROWS, N), f32, kind="ExternalInput")
    V0 = nc.dram_tensor("V0w", (3, 128, 128), f32, kind="ExternalInput")
    V1 = nc.dram_tensor("V1w", (3, 128, 128), f32, kind="ExternalInput")
    HS = nc.dram_tensor("HS", (128, 24), f32, kind="ExternalInput")
    Out = nc.dram_tensor("Xout", (RPC, N), f32, kind="ExternalOutput")

    with TileContext(nc) as tc:
        with (
            tc.tile_pool(name="const", bufs=1) as cpool,
            tc.tile_pool(name="io", bufs=3) as iopool,
            tc.tile_pool(name="g", bufs=2) as gpool,
            tc.tile_pool(name="w", bufs=2) as wpool,
            tc.tile_pool(name="ps", bufs=2, space="PSUM") as ppool,
        ):
            v0t = cpool.tile([128, 3 * 128], f32, tag="v0")
            v1t = cpool.tile([128, 3 * 128], f32, tag="v1")
            hst = cpool.tile([128, 24], f32, tag="hs")
            scr = cpool.tile([128, 4], f32, tag="scr")
            nc.sync.dma_start(hst[:, :], HS[:, :])
            for i in range(3):
                nc.sync.dma_start(v0t[:, i * 128:(i + 1) * 128], V0[i])
                nc.sync.dma_start(v1t[:, i * 128:(i + 1) * 128], V1[i])
            # consolidate const-DMA waits into the DVE clock once
            nc.vector.tensor_tensor(scr[:1, 0:1], hst[:1, 0:1], v0t[:1, 0:1],
                                    mybir.AluOpType.add)
            nc.vector.tensor_tensor(scr[:1, 1:2], hst[:1, 0:1], v1t[:1, 0:1],
                                    mybir.AluOpType.add)

            for ci, (r0, P) in enumerate(CHUNKS):
                hi = P - 10
                n_out = hi - OUT_LO
                orow = 108 * ci

                xt = iopool.tile([128, N], f32, tag="x")
                yt = iopool.tile([128, N], f32, tag="y")
                nc.sync.dma_start(xt[:P, :], Xc[r0:r0 + P, :])
                nc.sync.dma_start(yt[:P, :], yc[r0:r0 + P, :])

                g0 = gpool.tile([128, GW], f32, tag="g0")
                g1 = gpool.tile([128, GW], f32, tag="g1")
                w0 = wpool.tile([128, N], f32, tag="w0")
                w1 = wpool.tile([128, N], f32, tag="w1")
                ps = ppool.tile([128, N], f32, tag="ps")

                # absorb xt/yt DMA waits on the DVE clock (scan's ISA struct
                # has too few wait slots for Tile's cross-engine sems)
                nc.vector.tensor_tensor(w0[:1, 0:1], xt[:1, 0:1], yt[:1, 0:1],
                                        alu.add)
                nc.vector.memset(g0[:P, 0:G_PAD], 0.0)
                nc.vector.memset(g1[:P, 0:G_PAD], 0.0)

                # stage 0: g0 = cumsum(y - X) along rows
                nc.vector.tensor_tensor_scan(
                    g0[:P, G_PAD:GW], yt[:P, :], xt[:P, :], 0.0,
                    op0=alu.add, op1=alu.subtract,
                )
                # w0: 5-tap sums. interior, then right edge (2 cols), left scale
                nc.vector.tensor_tensor(
                    w0[:P, 0:2046], g0[:P, 11:GW], g0[:P, 6:2052], alu.subtract
                )
                nc.vector.scalar_tensor_tensor(
                    w0[:P, 2046:2048], g0[:P, 2052:2054], g0[:P, 2056:2057],
                    hst[:P, 2:4], op0=alu.subtract, op1=alu.mult,
                )
                nc.vector.tensor_tensor(
                    w0[:P, 0:2], w0[:P, 0:2], hst[:P, 0:2], alu.mult
                )
                for j in range(4):
                    sl = slice(j * 512, (j + 1) * 512)
                    nc.tensor.matmul(
                        ps[0:128, sl], v0t[0:P, ci * 128: ci * 128 + 128],
                        w0[:P, sl], start=True, stop=False, skip_group_check=True,
                    )
                # stage 1: g1 = g0 - cumsum(C1)
                nc.vector.tensor_tensor(w1[:1, 0:1], ps[:1, 0:1], g0[:1, 0:1],
                                        alu.add)
                nc.vector._custom_dve(
                    OP, out=g1[:P, G_PAD:GW], in0=g0[:P, G_PAD:GW], in1=ps[:P, 0:N]
                )
                nc.vector.tensor_tensor(
                    w1[:P, 0:2040], g1[:P, 17:GW], g1[:P, 0:2040], alu.subtract
                )
                nc.vector.scalar_tensor_tensor(
                    w1[:P, 2040:2048], g1[:P, 2040:2048], g1[:P, 2056:2057],
                    hst[:P, 12:20], op0=alu.subtract, op1=alu.mult,
                )
                nc.vector.tensor_tensor(
                    w1[:P, 0:8], w1[:P, 0:8], hst[:P, 4:12], alu.mult
                )
                for j in range(4):
                    sl = slice(j * 512, (j + 1) * 512)
                    nc.tensor.matmul(
                        ps[0:128, sl], v1t[0:P, ci * 128: ci * 128 + 128],
                        w1[:P, sl], start=False, stop=True, skip_group_check=True,
                    )
                # out = X + (C1 + C2)
                ot = iopool.tile([128, N], f32, tag="ot")
                o2 = iopool.tile([128, N], f32, tag="o2")
                nc.scalar.copy(ot[0:P, :], ps[0:P, 0:N])
                nc.gpsimd.tensor_tensor(
                    o2[0:P, :], ot[0:P, :], xt[0:P, :], alu.add
                )
                nc.sync.dma_start(Out[orow:orow + n_out, :], o2[OUT_LO:hi, :])
    nc.compile()
    return nc


def _host_inputs(X, y):
    """Per-core input maps. X, y: (2048, 2048) float32."""
    Xp = np.pad(X, ((HALO, HALO), (0, 0)))
    yp = np.pad(y, ((HALO, HALO), (0, 0)))

    def vcount(g, r):
        return np.minimum(g + r, M_DIM - 1) - np.maximum(g - r, 0) + 1

    rr = np.arange(128)
    band0 = (np.abs(rr[:, None] - rr[None, :]) <= 8).astype(np.float32)
    band1 = (np.abs(rr[:, None] - rr[None, :]) <= 2).astype(np.float32)

    hs = np.zeros(24, dtype=np.float32)
    hs[0:2] = [5.0 / 3.0, 5.0 / 4.0]
    hs[2:4] = [-5.0 / 4.0, -5.0 / 3.0]
    hs[4:12] = 17.0 / (9.0 + np.arange(8))
    hs[12:20] = -17.0 / (2056.0 - (2040.0 + np.arange(8)))
    HSt = np.tile(hs[None, :], (128, 1)).astype(np.float32)

    in_maps = []
    for k in range(NCORES):
        s = RPC * k
        V0w = np.zeros((3, 128, 128), dtype=np.float32)
        V1w = np.zeros((3, 128, 128), dtype=np.float32)
        for ci, (r0, P) in enumerate(CHUNKS):
            a = s - HALO + r0          # global row of local row 0
            m = np.arange(128)
            g = a + m
            valid = (g >= 0) & (g < M_DIM)
            gc = np.clip(g, 0, M_DIM - 1)
            m1lim = 120 if P == 128 else P - 8
            m2lim = 118 if P == 128 else P - 10
            mask1 = ((m >= 8) & (m < m1lim) & valid).astype(np.float32)
            mask2 = ((m >= OUT_LO) & (m < m2lim) & valid).astype(np.float32)
            sc0 = mask1 / (5.0 * vcount(gc, 8))
            sc1 = mask2 / (17.0 * vcount(gc, 2))
            V0w[ci] = band0 * sc0[None, :]
            V1w[ci] = band1 * sc1[None, :]
        in_maps.append({
            "Xc": np.ascontiguousarray(Xp[s:s + SRC_ROWS], dtype=np.float32),
            "yc": np.ascontiguousarray(yp[s:s + SRC_ROWS], dtype=np.float32),
            "V0w": V0w, "V1w": V1w, "HS": HSt,
        })
    return in_maps


def _run(X, y, trace=False):
    from concourse.bass_utils import run_bass_kernel_spmd

    if "nc" not in _CACHE:
        _CACHE["nc"] = _build_program()
    nc = _CACHE["nc"]
    in_maps = _host_inputs(X, y)
    res = run_bass_kernel_spmd(nc, in_maps, core_ids=list(range(NCORES)),
                               trace=trace)
    out = np.concatenate([r["Xout"] for r in res.results], axis=0)
    return out, res


def kernel(X, y, kernel):
    X2 = np.asarray(X, dtype=np.float32).reshape(M_DIM, N)
    y2 = np.asarray(y, dtype=np.float32).reshape(M_DIM, N)
    out, _ = _run(X2, y2)
    return out.reshape(1, 1, M_DIM, N)


# revision 8
# speedup vs baseline: 1.6036x; 1.6036x over previous
"""GuidedFilter (2-angle box guided filter) on 8 trn2 NeuronCores.

Math: for each stage s in {0, 1}:
    X <- X + box_s(y - X) / N_s
with box_0 = 17(rows) x 5(cols) ones kernel, box_1 = 5 x 17, zero-padded,
N_s the matching box filter of ones (separable: N_s = v_s(r) * h_s(c)).

Implementation per core (rows sharded, 256 rows/core, halo 10):
  3 independent row-chunks (128/128/60 source rows, stride 108).
  - g0 = rowwise cumsum(y - X)            (stock tensor_tensor_scan, DVE)
  - w0 = 5-tap window sums via shifted diffs of g0 (+ edge scale fixes)
  - C1 psum = V0w^T @ w0                  (TensorE; vertical 17-tap sum,
                                           normalizers folded into weights)
  - g1 = g0 - cumsum(C1)                  (custom DVE op: fused residual+scan)
  - w1 = 17-tap window sums of g1
  - psum += V1w^T @ w1                    (C1 + C2 accumulated in psum)
  - out = X + psum                        (ACT copy psum->sbuf, GPSIMD add)
"""

import sys

if "/opt/trn_rl_repo" not in sys.path:
    sys.path.insert(0, "/opt/trn_rl_repo")

import numpy as np

M_DIM = N = 2048
NCORES = 8
RPC = 256          # rows per core
HALO = 10
SRC_ROWS = RPC + 2 * HALO          # 276
CHUNKS = [(0, 128), (108, 128), (216, 60)]   # (local row start, rows)
OUT_LO = 10
G_PAD = 9
GW = G_PAD + N                     # 2057

_CACHE = {}


def _register_custom_op():
    from concourse.dve_spec import Spec, Src0, Src1, scan, AluOp, lower
    import concourse.dve_ops as dops
    from concourse.dve_uop import DveOpSpec

    name = "SUB_CUMSUM_GF"
    for op in dops.OPS:
        if op.name == name:
            return op
    spec = Spec(
        body=Src0 - scan(AluOp.ADD, Src1),
        reference=lambda in0, in1: in0 - np.cumsum(in1, axis=-1),
    )
    op = dops.DveOp(name, spec, subdim=False, uops_sha={})
    dops.OPS.append(op)
    dops.CUSTOM_DVE_SPECS[name] = spec
    dops._SUB_OPCODE_FOR_NAME[name] = max(dops._SUB_OPCODE_FOR_NAME.values()) + 1
    opc = dops.get_dve_sub_opcode(name)
    for ver in ("v3", "v4"):
        s = DveOpSpec(name=name, opcode=opc, uops=lower(spec, ver=ver), rd1_en=True)
        op.uops_sha[ver] = s.sha(ver)
    return op


def _build_program():
    from concourse import bacc
    import concourse.mybir as mybir
    from concourse.tile import TileContext

    OP = _register_custom_op()
    f32 = mybir.dt.float32
    alu = mybir.AluOpType

    nc = bacc.Bacc("TRN2", target_bir_lowering=False)
    Xc = nc.dram_tensor("Xc", (SRC_ROWS, N), f32, kind="ExternalInput")
    yc = nc.dram_tensor("yc", (SRC_ROWS, N), f32, kind="ExternalInput")
    fr = mybir.dt.float32r
    V0 = nc.dram_tensor("V0w", (3, 128, 128), fr, kind="ExternalInput")
    V1 = nc.dram_tensor("V1w", (3, 128, 128), fr, kind="ExternalInput")
    HS = nc.dram_tensor("HS", (128, 24), f32, kind="ExternalInput")
    Out = nc.dram_tensor("Xout", (RPC, N), f32, kind="ExternalOutput")

    with TileContext(nc) as tc:
        with (
            tc.tile_pool(name="const", bufs=1) as cpool,
            tc.tile_pool(name="io", bufs=3) as iopool,
            tc.tile_pool(name="g", bufs=2) as gpool,
            tc.tile_pool(name="w", bufs=2) as wpool,
            tc.tile_pool(name="ps", bufs=2, space="PSUM") as ppool,
        ):
            v0t = cpool.tile([128, 3 * 128], fr, tag="v0")
            v1t = cpool.tile([128, 3 * 128], fr, tag="v1")
            hst = cpool.tile([128, 24], f32, tag="hs")
            scr = cpool.tile([128, 4], f32, tag="scr")
            nc.sync.dma_start(hst[:, :], HS[:, :])
            for i in range(3):
                nc.sync.dma_start(v0t[:, i * 128:(i + 1) * 128], V0[i])
                nc.sync.dma_start(v1t[:, i * 128:(i + 1) * 128], V1[i])
            # consolidate const-DMA waits into the DVE clock once
            nc.vector.tensor_tensor(scr[:1, 0:1], hst[:1, 0:1], v0t[:1, 0:1],
                                    mybir.AluOpType.add)
            nc.vector.tensor_tensor(scr[:1, 1:2], hst[:1, 0:1], v1t[:1, 0:1],
                                    mybir.AluOpType.add)

            for ci, (r0, P) in enumerate(CHUNKS):
                hi = P - 10
                n_out = hi - OUT_LO
                orow = 108 * ci

                xt = iopool.tile([128, N], f32, tag="x")
                yt = iopool.tile([128, N], f32, tag="y")
                nc.sync.dma_start(xt[:P, :], Xc[r0:r0 + P, :])
                nc.sync.dma_start(yt[:P, :], yc[r0:r0 + P, :])

                g0 = gpool.tile([128, GW], f32, tag="g0")
                g1 = gpool.tile([128, GW], f32, tag="g1")
                w0 = wpool.tile([128, N], fr, tag="w0")
                w1 = wpool.tile([128, N], fr, tag="w1")
                ps = ppool.tile([128, N], f32, tag="ps")

                # absorb xt/yt DMA waits on the DVE clock (scan's ISA struct
                # has too few wait slots for Tile's cross-engine sems)
                nc.vector.tensor_tensor(w0[:1, 0:1], xt[:1, 0:1], yt[:1, 0:1],
                                        alu.add)
                nc.vector.memset(g0[:P, 0:G_PAD], 0.0)
                nc.vector.memset(g1[:P, 0:G_PAD], 0.0)

                # stage 0: g0 = cumsum(y - X) along rows
                nc.vector.tensor_tensor_scan(
                    g0[:P, G_PAD:GW], yt[:P, :], xt[:P, :], 0.0,
                    op0=alu.add, op1=alu.subtract,
                )
                # w0: 5-tap sums. interior, then right edge (2 cols), left scale
                nc.vector.tensor_tensor(
                    w0[:P, 0:2046], g0[:P, 11:GW], g0[:P, 6:2052], alu.subtract
                )
                nc.vector.scalar_tensor_tensor(
                    w0[:P, 2046:2048], g0[:P, 2052:2054], g0[:P, 2056:2057],
                    hst[:P, 2:4], op0=alu.subtract, op1=alu.mult,
                )
                nc.vector.tensor_tensor(
                    w0[:P, 0:2], w0[:P, 0:2], hst[:P, 0:2], alu.mult
                )
                for j in range(4):
                    sl = slice(j * 512, (j + 1) * 512)
                    nc.tensor.matmul(
                        ps[0:128, sl], v0t[0:P, ci * 128: ci * 128 + 128],
                        w0[:P, sl], start=True, stop=False, skip_group_check=True,
                    )
                # stage 1: g1 = g0 - cumsum(C1)
                nc.vector.tensor_tensor(w1[:1, 0:1], ps[:1, 0:1], g0[:1, 0:1],
                                        alu.add)
                nc.vector._custom_dve(
                    OP, out=g1[:P, G_PAD:GW], in0=g0[:P, G_PAD:GW], in1=ps[:P, 0:N]
                )
                nc.vector.tensor_tensor(
                    w1[:P, 0:2040], g1[:P, 17:GW], g1[:P, 0:2040], alu.subtract
                )
                nc.vector.scalar_tensor_tensor(
                    w1[:P, 2040:2048], g1[:P, 2040:2048], g1[:P, 2056:2057],
                    hst[:P, 12:20], op0=alu.subtract, op1=alu.mult,
                )
                nc.vector.tensor_tensor(
                    w1[:P, 0:8], w1[:P, 0:8], hst[:P, 4:12], alu.mult
                )
                for j in range(4):
                    sl = slice(j * 512, (j + 1) * 512)
                    nc.tensor.matmul(
                        ps[0:128, sl], v1t[0:P, ci * 128: ci * 128 + 128],
                        w1[:P, sl], start=False, stop=True, skip_group_check=True,
                    )
                # out = X + (C1 + C2)
                ot = iopool.tile([128, N], f32, tag="ot")
                o2 = iopool.tile([128, N], f32, tag="o2")
                nc.scalar.copy(ot[0:P, :], ps[0:P, 0:N])
                nc.gpsimd.tensor_tensor(
                    o2[0:P, :], ot[0:P, :], xt[0:P, :], alu.add
                )
                nc.sync.dma_start(Out[orow:orow + n_out, :], o2[OUT_LO:hi, :])
    nc.compile()
    return nc


def _host_inputs(X, y):
    """Per-core input maps. X, y: (2048, 2048) float32."""
    Xp = np.pad(X, ((HALO, HALO), (0, 0)))
    yp = np.pad(y, ((HALO, HALO), (0, 0)))

    def vcount(g, r):
        return np.minimum(g + r, M_DIM - 1) - np.maximum(g - r, 0) + 1

    rr = np.arange(128)
    band0 = (np.abs(rr[:, None] - rr[None, :]) <= 8).astype(np.float32)
    band1 = (np.abs(rr[:, None] - rr[None, :]) <= 2).astype(np.float32)

    hs = np.zeros(24, dtype=np.float32)
    hs[0:2] = [5.0 / 3.0, 5.0 / 4.0]
    hs[2:4] = [-5.0 / 4.0, -5.0 / 3.0]
    hs[4:12] = 17.0 / (9.0 + np.arange(8))
    hs[12:20] = -17.0 / (2056.0 - (2040.0 + np.arange(8)))
    HSt = np.tile(hs[None, :], (128, 1)).astype(np.float32)

    in_maps = []
    for k in range(NCORES):
        s = RPC * k
        V0w = np.zeros((3, 128, 128), dtype=np.float32)
        V1w = np.zeros((3, 128, 128), dtype=np.float32)
        for ci, (r0, P) in enumerate(CHUNKS):
            a = s - HALO + r0          # global row of local row 0
            m = np.arange(128)
            g = a + m
            valid = (g >= 0) & (g < M_DIM)
            gc = np.clip(g, 0, M_DIM - 1)
            m1lim = 120 if P == 128 else P - 8
            m2lim = 118 if P == 128 else P - 10
            mask1 = ((m >= 8) & (m < m1lim) & valid).astype(np.float32)
            mask2 = ((m >= OUT_LO) & (m < m2lim) & valid).astype(np.float32)
            sc0 = mask1 / (5.0 * vcount(gc, 8))
            sc1 = mask2 / (17.0 * vcount(gc, 2))
            V0w[ci] = band0 * sc0[None, :]
            V1w[ci] = band1 * sc1[None, :]
        in_maps.append({
            "Xc": np.ascontiguousarray(Xp[s:s + SRC_ROWS], dtype=np.float32),
            "yc": np.ascontiguousarray(yp[s:s + SRC_ROWS], dtype=np.float32),
            "V0w": V0w, "V1w": V1w, "HS": HSt,
        })
    return in_maps


def _run(X, y, trace=False):
    from concourse.bass_utils import run_bass_kernel_spmd

    if "nc" not in _CACHE:
        _CACHE["nc"] = _build_program()
    nc = _CACHE["nc"]
    in_maps = _host_inputs(X, y)
    res = run_bass_kernel_spmd(nc, in_maps, core_ids=list(range(NCORES)),
                               trace=trace)
    out = np.concatenate([r["Xout"] for r in res.results], axis=0)
    return out, res


def kernel(X, y, kernel):
    X2 = np.asarray(X, dtype=np.float32).reshape(M_DIM, N)
    y2 = np.asarray(y, dtype=np.float32).reshape(M_DIM, N)
    out, _ = _run(X2, y2)
    return out.reshape(1, 1, M_DIM, N)
